# revision 13
# baseline (speedup 1.0000x reference)
"""Trainium2 Bass kernel for GQA attention block with KV cache.

Computation (matches the reference):
    q = x @ Wq; k = x @ Wk; v = x @ Wv            (no bias)
    k, v = concat(past, new) along seq            (GQA: 8 kv heads, 32 q heads)
    out = softmax(q k^T / sqrt(hd) + mask) v
    out = out @ Wo

Sharding across 8 NeuronCores (one full TRN2 chip), done inside kernel():
  - Tensor-parallel over heads for projections + attention: core c owns
    q-heads 4c..4c+3 and kv head c (Wq/Wk/Wv column-sharded).
  - The attention output (still unnormalized-head-major) is exchanged with a
    single AllToAll so that each core ends up with ALL heads for 1/8 of the
    tokens, then does the out-projection token-sharded against the full Wo.
    This moves 8 MB per core instead of the 64 MB an all-reduce would.
  - Softmax denominators are folded in before the exchange, so no
    cross-core normalization state is needed.

All matmuls run as float32r (full PE rate, ~1.5e-4 rel err).  The mask input
is handled exactly: host-side, exp(mask) is classified per 512x128 block into
all-pass / all-blocked / partial; partial blocks are shipped as multiplicative
masks applied post-exp.  For the causal mask this skips ~30% of the score
blocks; for an all-zero mask it degrades to dense attention, still correct.
"""
import sys
import types
import numpy as np


def _ensure_axon_hooks():
    """antenv.axon_hooks may be absent; provide a stub so trace=True paths in
    bass_utils never crash on import.  Registers the real NTFF profiler hook
    when the boot helper is available (harmless otherwise)."""
    try:
        import antenv.axon_hooks  # noqa: F401
        return
    except Exception:
        pass
    mod = types.ModuleType("antenv.axon_hooks")
    mod._hook = None
    mod.set_axon_ntff_profile_hook = lambda h: setattr(mod, "_hook", h)
    mod.get_axon_ntff_profile_hook = lambda: mod._hook
    sys.modules["antenv.axon_hooks"] = mod
    try:
        import antenv
        antenv.axon_hooks = mod
    except Exception:
        pass
    try:
        from trn_agent_boot.trn_boot import _ntff_profile_via_ctypes
        mod._hook = _ntff_profile_via_ctypes("/opt/axon/libaxon_pjrt.so")
    except Exception:
        mod._hook = None


_ensure_axon_hooks()

import concourse.bacc as bacc
import concourse.mybir as mybir
import concourse.tile as tile
from concourse import bass_utils
from concourse.masks import make_identity

F32 = mybir.dt.float32
F32R = mybir.dt.float32r
AF = mybir.ActivationFunctionType

B, L, P, D = 2, 2048, 512, 4096
H, KH, HD = 32, 8, 128
S = P + L            # 2560 keys
W = 8                # cores
HPC = H // W         # 4 q heads per core
TQ = 512             # tokens per q-chunk (also the all-to-all chunk)
NQC = L // TQ        # 4 q chunks per sequence
NTOK = B * L         # 4096
NCH = NTOK // TQ     # 8 token chunks == W
ND = D // 128        # 32 contraction tiles for D
NS = S // 128        # 20 key tiles
ALPHA = 1.0 / float(HD) ** 0.5

LAST_RESULT = None   # BassKernelResults of the most recent run (for test harness)


def _mask_plan(mask):
    """mask: [L, S] additive attention mask (shared across batch/head).

    Returns (plan, mblocks): plan[qc] is a list of (key_tile, mblock_idx|None)
    to compute for queries [qc*TQ, (qc+1)*TQ); mblocks is [n, 128, TQ] float32,
    the exp(mask) of partial blocks transposed to [key, query] layout.
    exp is exact for the 0 / -1e9 masks (1.0 / 0.0)."""
    with np.errstate(over="ignore", under="ignore"):
        me = np.exp(mask.astype(np.float64)).astype(np.float32)
    plan = []
    blocks = []
    block_ids = {}
    for qc in range(NQC):
        row = []
        sub = me[qc * TQ:(qc + 1) * TQ]
        for st in range(NS):
            blk = sub[:, st * 128:(st + 1) * 128]
            if np.all(blk == 0.0):
                continue
            if np.all(blk == 1.0):
                row.append((st, None))
            else:
                bt = np.ascontiguousarray(blk.T)
                key = bt.tobytes()
                if key not in block_ids:
                    block_ids[key] = len(blocks)
                    blocks.append(bt)
                row.append((st, block_ids[key]))
        plan.append(row)
    mb = np.stack(blocks) if blocks else np.zeros((1, 128, TQ), np.float32)
    return plan, mb


def _build(plan, n_mb):
    nc = bacc.Bacc(None, target_bir_lowering=False, debug=False)
    xT = nc.dram_tensor("xT", [D, NTOK], F32R, kind="ExternalInput").ap()
    wq = nc.dram_tensor("wq", [D, HPC * HD], F32R, kind="ExternalInput").ap()
    wk = nc.dram_tensor("wk", [D, HD], F32R, kind="ExternalInput").ap()
    wv = nc.dram_tensor("wv", [D, HD], F32R, kind="ExternalInput").ap()
    pkT = nc.dram_tensor("pkT", [B, HD, P], F32R, kind="ExternalInput").ap()
    pv = nc.dram_tensor("pv", [B, P, HD], F32R, kind="ExternalInput").ap()
    wo = nc.dram_tensor("wo", [D, D], F32R, kind="ExternalInput").ap()
    mbk = nc.dram_tensor("mbk", [n_mb, 128, TQ], F32R, kind="ExternalInput").ap()
    out = nc.dram_tensor("out", [TQ, D], F32, kind="ExternalOutput").ap()

    # DRAM scratch
    qt_d = nc.dram_tensor("qt_d", [B, HPC, HD, L], F32R).ap()   # Q^T per head
    kt_d = nc.dram_tensor("kt_d", [B, HD, L], F32R).ap()        # new K^T
    v_d = nc.dram_tensor("v_d", [B, L, HD], F32R).ap()          # new V
    # all-to-all in two halves (heads 0-1, heads 2-3) so the first exchange
    # overlaps with the second half of the attention compute
    # first exchange carries heads 0-2 (fires while head 3 computes), second
    # carries head 3 and is hidden behind out-projection pass 0
    a2a_in = [nc.dram_tensor("a2a_in0", [NCH, 3 * HD, TQ], F32R),
              nc.dram_tensor("a2a_in1", [NCH, HD, TQ], F32R)]
    a2a_out = [nc.dram_tensor("a2a_out0", [NCH, 3 * HD, TQ], F32R),
               nc.dram_tensor("a2a_out1", [NCH, HD, TQ], F32R)]

    from contextlib import ExitStack
    with tile.TileContext(nc) as tc, nc.allow_low_precision("f32r matmul pipeline"):
        bstack = ExitStack()
        kvp = bstack.enter_context(tc.tile_pool(name="kvp", bufs=1))
        smb = bstack.enter_context(tc.tile_pool(name="smb", bufs=1))
        # Resident attention inputs (K^T, V, masks) allocated up front so their
        # loads overlap the projection phase instead of serializing behind it.
        ones_f = smb.tile([128, 1], F32, name="ones_f")
        nc.vector.memset(ones_f[:], 1.0)
        ones_s = smb.tile([128, 1], F32R, name="ones_s")
        nc.vector.tensor_copy(ones_s[:], ones_f[:])
        ones_1f = smb.tile([1, 128], F32, name="ones_1f")
        nc.vector.memset(ones_1f[:], 1.0)
        ones_1 = smb.tile([1, 128], F32R, name="ones_1")
        nc.vector.tensor_copy(ones_1[:], ones_1f[:])
        mb_t = smb.tile([128, n_mb, TQ], F32R, name="mb_t")
        nc.sync.dma_start(out=mb_t[:], in_=mbk.rearrange("n p t -> p n t"))
        ktbs, vbs = [], []
        for b in range(B):
            ktb = kvp.tile([128, S], F32R, name=f"ktb{b}", tag=f"ktb{b}")
            nc.sync.dma_start(out=ktb[:, 0:P], in_=pkT[b])
            vb = kvp.tile([128, NS, 128], F32R, name=f"vb{b}", tag=f"vb{b}")
            nc.sync.dma_start(out=vb[:, 0:P // 128, :],
                              in_=pv[b].rearrange("(st p) d -> p st d", p=128))
            ktbs.append(ktb)
            vbs.append(vb)

        # ---------------- Phase A: Q/K/V projections ----------------
        with tc.tile_pool(name="wpool", bufs=1) as wp, \
             tc.tile_pool(name="xkp", bufs=2) as xkp, \
             tc.tile_pool(name="evp", bufs=4) as evp, \
             tc.tile_pool(name="cstA", bufs=1) as cstA, \
             tc.tile_pool(name="psA", bufs=7, space="PSUM") as psA, \
             tc.tile_pool(name="pstr", bufs=1, space="PSUM") as pstr:
            identA = cstA.tile([128, 128], F32, name="identA")
            make_identity(nc, identA[:])
            wq_t = wp.tile([128, ND, HPC * HD], F32R, name="wq_t")
            wk_t = wp.tile([128, ND, HD], F32R, name="wk_t")
            wv_t = wp.tile([128, ND, HD], F32R, name="wv_t")
            wqr = wq.rearrange("(nd p) m -> p nd m", p=128)
            wkr = wk.rearrange("(nd p) m -> p nd m", p=128)
            wvr = wv.rearrange("(nd p) m -> p nd m", p=128)

            def load_weights(k0, k1):
                for k in range(k0, k1):
                    nc.sync.dma_start(out=wq_t[:, k, :], in_=wqr[:, k, :])
                    nc.sync.dma_start(out=wk_t[:, k, :], in_=wkr[:, k, :])
                    nc.sync.dma_start(out=wv_t[:, k, :], in_=wvr[:, k, :])

            xTr = xT.rearrange("(nd p) t -> p nd t", p=128)
            NQ4 = 4
            HF = ND // NQ4
            for tch in range(NCH):
                b, lc = tch // NQC, tch % NQC
                ps_list = [psA.tile([128, TQ], F32, name="psA_t", tag="psA_t")
                           for _ in range(6)]
                for hf in range(NQ4):
                    if tch == 0:
                        # feed the weight loads in lockstep with the first
                        # x chunk so the first matmul isn't stuck behind 12MB
                        # of weight DMA
                        load_weights(hf * HF, (hf + 1) * HF)
                    xk = xkp.tile([128, HF, TQ], F32R, name="xk", tag="xk")
                    nc.sync.dma_start(
                        out=xk[:],
                        in_=xTr[:, hf * HF:(hf + 1) * HF, tch * TQ:(tch + 1) * TQ])
                    for o in range(6):
                        for kk in range(HF):
                            k = hf * HF + kk
                            if o < 4:
                                lhsT = wq_t[:, k, o * 128:(o + 1) * 128]
                            elif o == 4:
                                lhsT = wk_t[:, k, :]
                            else:
                                lhsT = wv_t[:, k, :]
                            nc.tensor.matmul(ps_list[o][:], lhsT=lhsT,
                                             rhs=xk[:, kk, :],
                                             start=(k == 0), stop=(k == ND - 1))
                for o in range(4):
                    ev = evp.tile([128, TQ], F32R, name="ev", tag="ev")
                    nc.vector.tensor_copy(ev[:], ps_list[o][:])
                    nc.sync.dma_start(out=qt_d[b, o, :, lc * TQ:(lc + 1) * TQ], in_=ev[:])
                evk = evp.tile([128, TQ], F32R, name="evk", tag="ev")
                nc.vector.tensor_copy(evk[:], ps_list[4][:])
                nc.sync.dma_start(out=kt_d[b, :, lc * TQ:(lc + 1) * TQ], in_=evk[:])
                # V comes out of the projection transposed [d, s]; flip to [s, d]
                evv = evp.tile([128, TQ], F32, name="evv", tag="ev")
                nc.vector.tensor_copy(evv[:], ps_list[5][:])
                for i in range(4):
                    pt = pstr.tile([128, 128], F32, name="pt", tag="pt")
                    nc.tensor.transpose(pt[:], evv[:, i * 128:(i + 1) * 128], identA[:])
                    ev2 = evp.tile([128, 128], F32R, name="ev2", tag="ev2")
                    nc.vector.tensor_copy(ev2[:], pt[:])
                    nc.sync.dma_start(
                        out=v_d[b, lc * TQ + i * 128: lc * TQ + (i + 1) * 128, :],
                        in_=ev2[:])
                if lc == NQC - 1:
                    # this batch's K/V is complete; stage it for attention now
                    nc.sync.dma_start(out=ktbs[b][:, P:S], in_=kt_d[b])
                    nc.sync.dma_start(
                        out=vbs[b][:, P // 128:NS, :],
                        in_=v_d[b].rearrange("(st p) d -> p st d", p=128))

        # ---------------- Phase B: attention per (b, head, q-chunk) ----------------
        with tc.tile_pool(name="qtp", bufs=3) as qtp, \
             tc.tile_pool(name="esp", bufs=6) as esp, \
             tc.tile_pool(name="atp", bufs=3) as atp, \
             tc.tile_pool(name="psS", bufs=3, space="PSUM") as psS, \
             tc.tile_pool(name="psO", bufs=2, space="PSUM") as psO, \
             tc.tile_pool(name="psD", bufs=2, space="PSUM") as psD:
            pending = []  # deferred normalization of the previous chunk

            def flush_norm():
                if not pending:
                    return
                po, pd, b_, h_, qc_ = pending.pop()
                rd = atp.tile([1, TQ], F32R, name="rd", tag="rd")
                nc.vector.reciprocal(rd[:], pd[:])
                pb = psS.tile([128, TQ], F32, name="pb", tag="ps")
                nc.tensor.matmul(pb[:], lhsT=ones_1[:], rhs=rd[:])
                oev = atp.tile([128, TQ], F32, name="oev", tag="oev")
                nc.vector.tensor_copy(oev[:], po[:])
                at = atp.tile([128, TQ], F32R, name="at", tag="at")
                nc.vector.tensor_mul(at[:], oev[:], pb[:])
                half, hr = (0, h_) if h_ < 3 else (1, 0)
                nc.sync.dma_start(
                    out=a2a_in[half].ap()[b_ * NQC + qc_,
                                          hr * 128:(hr + 1) * 128, :],
                    in_=at[:])

            for h in range(HPC):
                for b in range(B):
                    ktb, vb = ktbs[b], vbs[b]
                    for qc in range(NQC):
                        qt = qtp.tile([128, TQ], F32R, name="qt", tag="qt")
                        nc.sync.dma_start(out=qt[:],
                                          in_=qt_d[b, h, :, qc * TQ:(qc + 1) * TQ])
                        po = psO.tile([128, TQ], F32, name="po", tag="po")
                        pd = psD.tile([1, TQ], F32, name="pd", tag="pd")
                        row = plan[qc]
                        for idx, (st, mb) in enumerate(row):
                            ps = psS.tile([128, TQ], F32, name="ps", tag="ps")
                            nc.tensor.matmul(ps[:],
                                             lhsT=ktb[:, st * 128:(st + 1) * 128],
                                             rhs=qt[:])
                            es = esp.tile([128, TQ], F32R, name="es", tag="es")
                            nc.scalar.activation(es[:], ps[:], AF.Exp, scale=ALPHA)
                            if mb is not None:
                                es2 = esp.tile([128, TQ], F32R, name="es2", tag="es")
                                nc.vector.tensor_mul(es2[:], es[:], mb_t[:, mb, :])
                                es = es2
                            first, last = (idx == 0), (idx == len(row) - 1)
                            nc.tensor.matmul(po[:], lhsT=vb[:, st, :], rhs=es[:],
                                             start=first, stop=last)
                            nc.tensor.matmul(pd[:], lhsT=ones_s[:], rhs=es[:],
                                             start=first, stop=last)
                            if idx == 3:
                                flush_norm()  # previous chunk, now safely overlapped
                        pending.append((po, pd, b, h, qc))
                if h == 2:
                    flush_norm()
                    nc.gpsimd.collective_compute(
                        "AllToAll", mybir.AluOpType.bypass,
                        ins=[a2a_in[0].ap()], outs=[a2a_out[0].ap()],
                        replica_groups=[list(range(W))])
            flush_norm()
            nc.gpsimd.collective_compute(
                "AllToAll", mybir.AluOpType.bypass,
                ins=[a2a_in[1].ap()], outs=[a2a_out[1].ap()],
                replica_groups=[list(range(W))])

        bstack.close()  # release K/V/mask residency before the out-projection
        # ---------------- Phase C: out projection, token-sharded ----------------
        # Two passes: pass 0 consumes the early all-to-all half (heads 0-1 of
        # every core) and banks partial sums in SBUF; pass 1 adds the rest.
        with tc.tile_pool(name="a2ap", bufs=1) as a2ap, \
             tc.tile_pool(name="accp", bufs=1) as accp, \
             tc.tile_pool(name="wop", bufs=26) as wop, \
             tc.tile_pool(name="evC", bufs=4) as evC, \
             tc.tile_pool(name="psC", bufs=8, space="PSUM") as psC:
            asb = a2ap.tile([128, H, TQ], F32R, name="asb")
            for w in range(W):
                for hh in range(3):
                    nc.sync.dma_start(
                        out=asb[:, w * HPC + hh, :],
                        in_=a2a_out[0].ap()[w, hh * 128:(hh + 1) * 128, :])
            for w in range(W):
                nc.sync.dma_start(
                    out=asb[:, w * HPC + 3, :],
                    in_=a2a_out[1].ap()[w, 0:128, :])
            accs = [accp.tile([128, D], F32, name=f"acc{tt}", tag=f"acc{tt}")
                    for tt in range(TQ // 128)]
            wor = wo.rearrange("(nh p) dd -> p nh dd", p=128)
            for pas in range(2):
                if pas == 0:
                    hts = [w * HPC + hh for w in range(W) for hh in range(3)]
                else:
                    hts = [w * HPC + 3 for w in range(W)]
                for dc in range(D // 512):
                    pcs = [psC.tile([128, 512], F32, name="pc", tag="pc")
                           for _ in range(TQ // 128)]
                    for i, ht in enumerate(hts):
                        wt = wop.tile([128, 512], F32R, name="wt", tag="wt")
                        nc.sync.dma_start(out=wt[:],
                                          in_=wor[:, ht, dc * 512:(dc + 1) * 512])
                        for tt in range(TQ // 128):
                            nc.tensor.matmul(pcs[tt][:],
                                             lhsT=asb[:, ht, tt * 128:(tt + 1) * 128],
                                             rhs=wt[:],
                                             start=(i == 0), stop=(i == len(hts) - 1))
                    for tt in range(TQ // 128):
                        if pas == 0:
                            nc.vector.tensor_copy(
                                accs[tt][:, dc * 512:(dc + 1) * 512], pcs[tt][:])
                        else:
                            evc = evC.tile([128, 512], F32, name="evc", tag="evc")
                            nc.vector.tensor_add(
                                evc[:], pcs[tt][:],
                                accs[tt][:, dc * 512:(dc + 1) * 512])
                            nc.sync.dma_start(
                                out=out[tt * 128:(tt + 1) * 128,
                                        dc * 512:(dc + 1) * 512],
                                in_=evc[:])

    nc.compile()
    return nc


def kernel(**inputs):
    global LAST_RESULT
    x = np.asarray(inputs["x"], np.float32)
    mask = np.asarray(inputs["mask"], np.float32)[0, 0]
    past_k = np.asarray(inputs["past_k"], np.float32)
    past_v = np.asarray(inputs["past_v"], np.float32)
    Wq = np.asarray(inputs["Wq"], np.float32)
    Wk = np.asarray(inputs["Wk"], np.float32)
    Wv = np.asarray(inputs["Wv"], np.float32)
    Wo = np.asarray(inputs["Wo"], np.float32)

    plan, mb = _mask_plan(mask)
    nc = _build(plan, mb.shape[0])

    xT = np.ascontiguousarray(x.reshape(NTOK, D).T)
    in_maps = []
    for c in range(W):
        in_maps.append({
            "xT": xT,
            "wq": np.ascontiguousarray(Wq[:, c * HPC * HD:(c + 1) * HPC * HD]),
            "wk": np.ascontiguousarray(Wk[:, c * HD:(c + 1) * HD]),
            "wv": np.ascontiguousarray(Wv[:, c * HD:(c + 1) * HD]),
            "pkT": np.ascontiguousarray(past_k[:, c].transpose(0, 2, 1)),
            "pv": np.ascontiguousarray(past_v[:, c]),
            "wo": Wo,
            "mbk": mb,
        })
    res = None
    for attempt in range(3):
        try:
            res = bass_utils.run_bass_kernel_spmd(nc, in_maps, list(range(W)))
            break
        except Exception:
            if attempt == 2:
                raise
            import time as _time
            try:
                import jax as _jax
                _jax.clear_caches()
            except Exception:
                pass
            _time.sleep(3)
    LAST_RESULT = res
    out = np.empty((B, L, D), np.float32)
    for c in range(W):
        b, qc = c // NQC, c % NQC
        out[b, qc * TQ:(qc + 1) * TQ] = res.results[c]["out"]
    return out


# revision 14
# speedup vs baseline: 1.0343x; 1.0343x over previous
"""Trainium2 Bass kernel for GQA attention block with KV cache.

Computation (matches the reference):
    q = x @ Wq; k = x @ Wk; v = x @ Wv            (no bias)
    k, v = concat(past, new) along seq            (GQA: 8 kv heads, 32 q heads)
    out = softmax(q k^T / sqrt(hd) + mask) v
    out = out @ Wo

Sharding across 8 NeuronCores (one full TRN2 chip), done inside kernel():
  - Tensor-parallel over heads for projections + attention: core c owns
    q-heads 4c..4c+3 and kv head c (Wq/Wk/Wv column-sharded).
  - The attention output (still unnormalized-head-major) is exchanged with a
    single AllToAll so that each core ends up with ALL heads for 1/8 of the
    tokens, then does the out-projection token-sharded against the full Wo.
    This moves 8 MB per core instead of the 64 MB an all-reduce would.
  - Softmax denominators are folded in before the exchange, so no
    cross-core normalization state is needed.

All matmuls run as float32r (full PE rate, ~1.5e-4 rel err).  The mask input
is handled exactly: host-side, exp(mask) is classified per 512x128 block into
all-pass / all-blocked / partial; partial blocks are shipped as multiplicative
masks applied post-exp.  For the causal mask this skips ~30% of the score
blocks; for an all-zero mask it degrades to dense attention, still correct.
"""
import sys
import types
import numpy as np


def _ensure_axon_hooks():
    """antenv.axon_hooks may be absent; provide a stub so trace=True paths in
    bass_utils never crash on import.  Registers the real NTFF profiler hook
    when the boot helper is available (harmless otherwise)."""
    try:
        import antenv.axon_hooks  # noqa: F401
        return
    except Exception:
        pass
    mod = types.ModuleType("antenv.axon_hooks")
    mod._hook = None
    mod.set_axon_ntff_profile_hook = lambda h: setattr(mod, "_hook", h)
    mod.get_axon_ntff_profile_hook = lambda: mod._hook
    sys.modules["antenv.axon_hooks"] = mod
    try:
        import antenv
        antenv.axon_hooks = mod
    except Exception:
        pass
    try:
        from trn_agent_boot.trn_boot import _ntff_profile_via_ctypes
        mod._hook = _ntff_profile_via_ctypes("/opt/axon/libaxon_pjrt.so")
    except Exception:
        mod._hook = None


_ensure_axon_hooks()

import concourse.bacc as bacc
import concourse.mybir as mybir
import concourse.tile as tile
from concourse import bass_utils
from concourse.masks import make_identity

F32 = mybir.dt.float32
F32R = mybir.dt.float32r
AF = mybir.ActivationFunctionType

B, L, P, D = 2, 2048, 512, 4096
H, KH, HD = 32, 8, 128
S = P + L            # 2560 keys
W = 8                # cores
HPC = H // W         # 4 q heads per core
TQ = 512             # tokens per q-chunk (also the all-to-all chunk)
NQC = L // TQ        # 4 q chunks per sequence
NTOK = B * L         # 4096
NCH = NTOK // TQ     # 8 token chunks == W
ND = D // 128        # 32 contraction tiles for D
NS = S // 128        # 20 key tiles
ALPHA = 1.0 / float(HD) ** 0.5

LAST_RESULT = None   # BassKernelResults of the most recent run (for test harness)


def _mask_plan(mask):
    """mask: [L, S] additive attention mask (shared across batch/head).

    Returns (plan, mblocks): plan[qc] is a list of (key_tile, mblock_idx|None)
    to compute for queries [qc*TQ, (qc+1)*TQ); mblocks is [n, 128, TQ] float32,
    the exp(mask) of partial blocks transposed to [key, query] layout.
    exp is exact for the 0 / -1e9 masks (1.0 / 0.0)."""
    with np.errstate(over="ignore", under="ignore"):
        me = np.exp(mask.astype(np.float64)).astype(np.float32)
    plan = []
    blocks = []
    block_ids = {}
    for qc in range(NQC):
        row = []
        sub = me[qc * TQ:(qc + 1) * TQ]
        for st in range(NS):
            blk = sub[:, st * 128:(st + 1) * 128]
            if np.all(blk == 0.0):
                continue
            if np.all(blk == 1.0):
                row.append((st, None))
            else:
                bt = np.ascontiguousarray(blk.T)
                key = bt.tobytes()
                if key not in block_ids:
                    block_ids[key] = len(blocks)
                    blocks.append(bt)
                row.append((st, block_ids[key]))
        plan.append(row)
    mb = np.stack(blocks) if blocks else np.zeros((1, 128, TQ), np.float32)
    return plan, mb


def _build(plan, n_mb):
    nc = bacc.Bacc(None, target_bir_lowering=False, debug=False)
    xT = nc.dram_tensor("xT", [D, NTOK], F32R, kind="ExternalInput").ap()
    wq = nc.dram_tensor("wq", [D, HPC * HD], F32R, kind="ExternalInput").ap()
    wk = nc.dram_tensor("wk", [D, HD], F32R, kind="ExternalInput").ap()
    wv = nc.dram_tensor("wv", [D, HD], F32R, kind="ExternalInput").ap()
    pkT = nc.dram_tensor("pkT", [B, HD, P], F32R, kind="ExternalInput").ap()
    pv = nc.dram_tensor("pv", [B, P, HD], F32R, kind="ExternalInput").ap()
    wo = nc.dram_tensor("wo", [D, D], F32R, kind="ExternalInput").ap()
    mbk = nc.dram_tensor("mbk", [n_mb, 128, TQ], F32R, kind="ExternalInput").ap()
    out = nc.dram_tensor("out", [TQ, D], F32, kind="ExternalOutput").ap()

    # DRAM scratch
    qt_d = nc.dram_tensor("qt_d", [B, HPC, HD, L], F32R).ap()   # Q^T per head
    kt_d = nc.dram_tensor("kt_d", [B, HD, L], F32R).ap()        # new K^T
    v_d = nc.dram_tensor("v_d", [B, L, HD], F32R).ap()          # new V
    # all-to-all in two halves (heads 0-1, heads 2-3) so the first exchange
    # overlaps with the second half of the attention compute
    # first exchange carries heads 0-2 (fires while head 3 computes), second
    # carries head 3 and is hidden behind out-projection pass 0
    a2a_in = [nc.dram_tensor("a2a_in0", [NCH, 2 * HD, TQ], F32R),
              nc.dram_tensor("a2a_in1", [NCH, 2 * HD, TQ], F32R)]
    a2a_out = [nc.dram_tensor("a2a_out0", [NCH, 2 * HD, TQ], F32R),
               nc.dram_tensor("a2a_out1", [NCH, 2 * HD, TQ], F32R)]

    from contextlib import ExitStack
    with tile.TileContext(nc) as tc, nc.allow_low_precision("f32r matmul pipeline"):
        bstack = ExitStack()
        kvp = bstack.enter_context(tc.tile_pool(name="kvp", bufs=1))
        smb = bstack.enter_context(tc.tile_pool(name="smb", bufs=1))
        # Resident attention inputs (K^T, V, masks) allocated up front so their
        # loads overlap the projection phase instead of serializing behind it.
        ones_f = smb.tile([128, 1], F32, name="ones_f")
        nc.vector.memset(ones_f[:], 1.0)
        ones_s = smb.tile([128, 1], F32R, name="ones_s")
        nc.vector.tensor_copy(ones_s[:], ones_f[:])
        ones_1f = smb.tile([1, 128], F32, name="ones_1f")
        nc.vector.memset(ones_1f[:], 1.0)
        ones_1 = smb.tile([1, 128], F32R, name="ones_1")
        nc.vector.tensor_copy(ones_1[:], ones_1f[:])
        mb_t = smb.tile([128, n_mb, TQ], F32R, name="mb_t")
        nc.sync.dma_start(out=mb_t[:], in_=mbk.rearrange("n p t -> p n t"))
        ktbs, vbs = [], []
        for b in range(B):
            ktb = kvp.tile([128, S], F32R, name=f"ktb{b}", tag=f"ktb{b}")
            nc.sync.dma_start(out=ktb[:, 0:P], in_=pkT[b])
            vb = kvp.tile([128, NS, 128], F32R, name=f"vb{b}", tag=f"vb{b}")
            nc.sync.dma_start(out=vb[:, 0:P // 128, :],
                              in_=pv[b].rearrange("(st p) d -> p st d", p=128))
            ktbs.append(ktb)
            vbs.append(vb)

        # ---------------- Phase A: Q/K/V projections ----------------
        with tc.tile_pool(name="wpool", bufs=1) as wp, \
             tc.tile_pool(name="xkp", bufs=2) as xkp, \
             tc.tile_pool(name="evp", bufs=4) as evp, \
             tc.tile_pool(name="cstA", bufs=1) as cstA, \
             tc.tile_pool(name="psA", bufs=7, space="PSUM") as psA, \
             tc.tile_pool(name="pstr", bufs=1, space="PSUM") as pstr:
            identA = cstA.tile([128, 128], F32, name="identA")
            make_identity(nc, identA[:])
            wq_t = wp.tile([128, ND, HPC * HD], F32R, name="wq_t")
            wk_t = wp.tile([128, ND, HD], F32R, name="wk_t")
            wv_t = wp.tile([128, ND, HD], F32R, name="wv_t")
            wqr = wq.rearrange("(nd p) m -> p nd m", p=128)
            wkr = wk.rearrange("(nd p) m -> p nd m", p=128)
            wvr = wv.rearrange("(nd p) m -> p nd m", p=128)

            def load_weights(k0, k1):
                for k in range(k0, k1):
                    nc.sync.dma_start(out=wq_t[:, k, :], in_=wqr[:, k, :])
                    nc.sync.dma_start(out=wk_t[:, k, :], in_=wkr[:, k, :])
                    nc.sync.dma_start(out=wv_t[:, k, :], in_=wvr[:, k, :])

            xTr = xT.rearrange("(nd p) t -> p nd t", p=128)
            NQ4 = 4
            HF = ND // NQ4
            for tch in range(NCH):
                b, lc = tch // NQC, tch % NQC
                ps_list = [psA.tile([128, TQ], F32, name="psA_t", tag="psA_t")
                           for _ in range(6)]
                for hf in range(NQ4):
                    if tch == 0:
                        # feed the weight loads in lockstep with the first
                        # x chunk so the first matmul isn't stuck behind 12MB
                        # of weight DMA
                        load_weights(hf * HF, (hf + 1) * HF)
                    xk = xkp.tile([128, HF, TQ], F32R, name="xk", tag="xk")
                    nc.sync.dma_start(
                        out=xk[:],
                        in_=xTr[:, hf * HF:(hf + 1) * HF, tch * TQ:(tch + 1) * TQ])
                    for o in range(6):
                        for kk in range(HF):
                            k = hf * HF + kk
                            if o < 4:
                                lhsT = wq_t[:, k, o * 128:(o + 1) * 128]
                            elif o == 4:
                                lhsT = wk_t[:, k, :]
                            else:
                                lhsT = wv_t[:, k, :]
                            nc.tensor.matmul(ps_list[o][:], lhsT=lhsT,
                                             rhs=xk[:, kk, :],
                                             start=(k == 0), stop=(k == ND - 1))
                for o in range(4):
                    ev = evp.tile([128, TQ], F32R, name="ev", tag="ev")
                    nc.vector.tensor_copy(ev[:], ps_list[o][:])
                    nc.sync.dma_start(out=qt_d[b, o, :, lc * TQ:(lc + 1) * TQ], in_=ev[:])
                evk = evp.tile([128, TQ], F32R, name="evk", tag="ev")
                nc.vector.tensor_copy(evk[:], ps_list[4][:])
                nc.sync.dma_start(out=kt_d[b, :, lc * TQ:(lc + 1) * TQ], in_=evk[:])
                # V comes out of the projection transposed [d, s]; flip to [s, d]
                evv = evp.tile([128, TQ], F32, name="evv", tag="ev")
                nc.vector.tensor_copy(evv[:], ps_list[5][:])
                for i in range(4):
                    pt = pstr.tile([128, 128], F32, name="pt", tag="pt")
                    nc.tensor.transpose(pt[:], evv[:, i * 128:(i + 1) * 128], identA[:])
                    ev2 = evp.tile([128, 128], F32R, name="ev2", tag="ev2")
                    nc.vector.tensor_copy(ev2[:], pt[:])
                    nc.sync.dma_start(
                        out=v_d[b, lc * TQ + i * 128: lc * TQ + (i + 1) * 128, :],
                        in_=ev2[:])
                if lc == NQC - 1:
                    # this batch's K/V is complete; stage it for attention now
                    nc.sync.dma_start(out=ktbs[b][:, P:S], in_=kt_d[b])
                    nc.sync.dma_start(
                        out=vbs[b][:, P // 128:NS, :],
                        in_=v_d[b].rearrange("(st p) d -> p st d", p=128))

        # ---------------- Phase B: attention per (b, head, q-chunk) ----------------
        with tc.tile_pool(name="qtp", bufs=3) as qtp, \
             tc.tile_pool(name="esp", bufs=6) as esp, \
             tc.tile_pool(name="atp", bufs=3) as atp, \
             tc.tile_pool(name="psS", bufs=3, space="PSUM") as psS, \
             tc.tile_pool(name="psO", bufs=2, space="PSUM") as psO, \
             tc.tile_pool(name="psD", bufs=2, space="PSUM") as psD:
            pending = []  # deferred normalization of the previous chunk

            def flush_norm():
                if not pending:
                    return
                po, pd, b_, h_, qc_ = pending.pop()
                rd = atp.tile([1, TQ], F32R, name="rd", tag="rd")
                nc.vector.reciprocal(rd[:], pd[:])
                pb = psS.tile([128, TQ], F32, name="pb", tag="ps")
                nc.tensor.matmul(pb[:], lhsT=ones_1[:], rhs=rd[:])
                oev = atp.tile([128, TQ], F32, name="oev", tag="oev")
                nc.vector.tensor_copy(oev[:], po[:])
                at = atp.tile([128, TQ], F32R, name="at", tag="at")
                nc.vector.tensor_mul(at[:], oev[:], pb[:])
                half, hr = h_ // 2, h_ % 2
                nc.sync.dma_start(
                    out=a2a_in[half].ap()[b_ * NQC + qc_,
                                          hr * 128:(hr + 1) * 128, :],
                    in_=at[:])

            for h in range(HPC):
                for b in range(B):
                    ktb, vb = ktbs[b], vbs[b]
                    for qc in range(NQC):
                        qt = qtp.tile([128, TQ], F32R, name="qt", tag="qt")
                        nc.sync.dma_start(out=qt[:],
                                          in_=qt_d[b, h, :, qc * TQ:(qc + 1) * TQ])
                        po = psO.tile([128, TQ], F32, name="po", tag="po")
                        pd = psD.tile([1, TQ], F32, name="pd", tag="pd")
                        row = plan[qc]
                        for idx, (st, mb) in enumerate(row):
                            ps = psS.tile([128, TQ], F32, name="ps", tag="ps")
                            nc.tensor.matmul(ps[:],
                                             lhsT=ktb[:, st * 128:(st + 1) * 128],
                                             rhs=qt[:])
                            es = esp.tile([128, TQ], F32R, name="es", tag="es")
                            nc.scalar.activation(es[:], ps[:], AF.Exp, scale=ALPHA)
                            if mb is not None:
                                es2 = esp.tile([128, TQ], F32R, name="es2", tag="es")
                                nc.vector.tensor_mul(es2[:], es[:], mb_t[:, mb, :])
                                es = es2
                            first, last = (idx == 0), (idx == len(row) - 1)
                            nc.tensor.matmul(po[:], lhsT=vb[:, st, :], rhs=es[:],
                                             start=first, stop=last)
                            nc.tensor.matmul(pd[:], lhsT=ones_s[:], rhs=es[:],
                                             start=first, stop=last)
                            if idx == 3:
                                flush_norm()  # previous chunk, now safely overlapped
                        pending.append((po, pd, b, h, qc))
                if h == 1:
                    flush_norm()
                    nc.gpsimd.collective_compute(
                        "AllToAll", mybir.AluOpType.bypass,
                        ins=[a2a_in[0].ap()], outs=[a2a_out[0].ap()],
                        replica_groups=[list(range(W))])
            flush_norm()
            nc.gpsimd.collective_compute(
                "AllToAll", mybir.AluOpType.bypass,
                ins=[a2a_in[1].ap()], outs=[a2a_out[1].ap()],
                replica_groups=[list(range(W))])

        bstack.close()  # release K/V/mask residency before the out-projection
        # ---------------- Phase C: out projection, token-sharded ----------------
        # Two passes: pass 0 consumes the early all-to-all half (heads 0-1 of
        # every core) and banks partial sums in SBUF; pass 1 adds the rest.
        with tc.tile_pool(name="a2ap", bufs=1) as a2ap, \
             tc.tile_pool(name="accp", bufs=1) as accp, \
             tc.tile_pool(name="wop", bufs=26) as wop, \
             tc.tile_pool(name="evC", bufs=4) as evC, \
             tc.tile_pool(name="psC", bufs=8, space="PSUM") as psC:
            asb = a2ap.tile([128, H, TQ], F32R, name="asb")
            for half in range(2):
                for w in range(W):
                    for hh in range(2):
                        nc.sync.dma_start(
                            out=asb[:, w * HPC + half * 2 + hh, :],
                            in_=a2a_out[half].ap()[w, hh * 128:(hh + 1) * 128, :])
            accs = [accp.tile([128, D], F32, name=f"acc{tt}", tag=f"acc{tt}")
                    for tt in range(TQ // 128)]
            wor = wo.rearrange("(nh p) dd -> p nh dd", p=128)
            for pas in range(2):
                hts = [w * HPC + pas * 2 + hh for w in range(W) for hh in range(2)]
                for dc in range(D // 512):
                    pcs = [psC.tile([128, 512], F32, name="pc", tag="pc")
                           for _ in range(TQ // 128)]
                    for i, ht in enumerate(hts):
                        wt = wop.tile([128, 512], F32R, name="wt", tag="wt")
                        nc.sync.dma_start(out=wt[:],
                                          in_=wor[:, ht, dc * 512:(dc + 1) * 512])
                        for tt in range(TQ // 128):
                            nc.tensor.matmul(pcs[tt][:],
                                             lhsT=asb[:, ht, tt * 128:(tt + 1) * 128],
                                             rhs=wt[:],
                                             start=(i == 0), stop=(i == len(hts) - 1))
                    for tt in range(TQ // 128):
                        if pas == 0:
                            nc.vector.tensor_copy(
                                accs[tt][:, dc * 512:(dc + 1) * 512], pcs[tt][:])
                        else:
                            evc = evC.tile([128, 512], F32, name="evc", tag="evc")
                            nc.vector.tensor_add(
                                evc[:], pcs[tt][:],
                                accs[tt][:, dc * 512:(dc + 1) * 512])
                            nc.sync.dma_start(
                                out=out[tt * 128:(tt + 1) * 128,
                                        dc * 512:(dc + 1) * 512],
                                in_=evc[:])

    nc.compile()
    return nc


def kernel(**inputs):
    global LAST_RESULT
    x = np.asarray(inputs["x"], np.float32)
    mask = np.asarray(inputs["mask"], np.float32)[0, 0]
    past_k = np.asarray(inputs["past_k"], np.float32)
    past_v = np.asarray(inputs["past_v"], np.float32)
    Wq = np.asarray(inputs["Wq"], np.float32)
    Wk = np.asarray(inputs["Wk"], np.float32)
    Wv = np.asarray(inputs["Wv"], np.float32)
    Wo = np.asarray(inputs["Wo"], np.float32)

    plan, mb = _mask_plan(mask)
    nc = _build(plan, mb.shape[0])

    xT = np.ascontiguousarray(x.reshape(NTOK, D).T)
    in_maps = []
    for c in range(W):
        in_maps.append({
            "xT": xT,
            "wq": np.ascontiguousarray(Wq[:, c * HPC * HD:(c + 1) * HPC * HD]),
            "wk": np.ascontiguousarray(Wk[:, c * HD:(c + 1) * HD]),
            "wv": np.ascontiguousarray(Wv[:, c * HD:(c + 1) * HD]),
            "pkT": np.ascontiguousarray(past_k[:, c].transpose(0, 2, 1)),
            "pv": np.ascontiguousarray(past_v[:, c]),
            "wo": Wo,
            "mbk": mb,
        })
    res = None
    for attempt in range(3):
        try:
            res = bass_utils.run_bass_kernel_spmd(nc, in_maps, list(range(W)))
            break
        except Exception:
            if attempt == 2:
                raise
            import time as _time
            try:
                import jax as _jax
                _jax.clear_caches()
            except Exception:
                pass
            _time.sleep(3)
    LAST_RESULT = res
    out = np.empty((B, L, D), np.float32)
    for c in range(W):
        b, qc = c // NQC, c % NQC
        out[b, qc * TQ:(qc + 1) * TQ] = res.results[c]["out"]
    return out
